# revision 28
# baseline (speedup 1.0000x reference)
"""GNN message passing (u_mul_e -> segment_sum) on 8 Trainium2 NeuronCores.

out[v] = sum_{e=(u->v)} h[u] * w[e]

Strategy (edge/graph parallelism, dst-range sharded -> no collectives):
  - Each core owns a contiguous range of 12500 destination nodes.
  - Host buckets edges by (core, 128-node dst strip, src chunk), sorts, and
    pads each bucket to a multiple of 128 edges, uniformly across cores
    (same instruction stream on every core = SPMD).
  - Device, per group of SPG=5 strips (20 pipeline groups, double-buffered):
      * dma_gather pulls h rows (bf16, padded to 256B) for each edge's src —
        one 256B descriptor per edge, one un-split gather per (group, chunk),
        spread over all 4 SWDGE queues (queue = chunk % 4). The gather is the
        bottleneck: measured ~3.0 ns/descriptor at 4 queues (descriptor-count
        proportional; random src is FASTER than repeated rows since it
        spreads across HBM banks).
      * DVE builds a weighted one-hot matrix P[e, j] = (offs_e == j) per
        128-edge block in bulk (is_equal against a broadcast iota, Act does
        the offs broadcast), and multiplies gathered h rows by w. Bulk ops
        keep per-tile dependency waits off the DVE sequencer so compute
        hides entirely under the gather.
      * PE computes P^T @ msg per strip, accumulating the [128, 32] segment
        sums in PSUM across the strip's blocks.
  - Per-strip results are copied to an SBUF output buffer and DMA'd out once.

Perf knobs (env, defaults tuned): K_SPG=5, K_SWDGEQ=4, K_SUBG=0.
Measured: ~421 us/exec steady-state (median interleaved-pair slope, 21-rep
device-unrolled), ~600 us/exec with a full barrier between reps (For_i loop
mode, K_LOOP=1); baseline was 841 us. Alternative gather modes kept for
reference: K_PACK4 (4 nodes per 256B row, 15% fewer descriptors but its
per-tile DVE chain exposes ~600 us), K_GMODE=ind (indirect/dynamic-DGE
64B-row gather — faster on paper but overflows the 1024-descriptor dynamic
ring and desyncs the device; do not use).
"""

import os
import sys

sys.path.insert(0, "/opt/trn_rl_repo")

import numpy as np
import ml_dtypes

BF16 = ml_dtypes.bfloat16

# Full-problem configuration (hardcoded; kernel.py must be self-contained).
# Tuned: SPG=5 (smaller pipeline groups overlap gather/compute best) and
# 4 SWDGE queues with one un-split gather per (group, chunk) run.
FULL_CFG = dict(
    N=100000,   # nodes
    E=1600000,  # edges
    D=32,       # feature dim
    NC=8,       # cores
    CH=4,       # src chunks (N/CH must be <= 32767 for int16 gather indices)
    SPG=int(os.environ.get("K_SPG", "5")),  # dst strips (128 nodes each) per pipeline group
)


def _derive(cfg):
    c = dict(cfg)
    c["PACK4"] = bool(int(os.environ.get("K_PACK4", "0")))
    c["IND"] = os.environ.get("K_GMODE", "swdge") == "ind"
    if c["IND"]:
        c["PACK4"] = False
        c["CH"] = 1
    if c["PACK4"]:
        c["CH"] = 1
    assert c["N"] % c["NC"] == 0
    c["NPC"] = c["N"] // c["NC"]               # dst nodes per core
    c["S"] = -(-c["NPC"] // 128)               # strips per core
    assert c["N"] % c["CH"] == 0
    c["CHN"] = c["N"] // c["CH"]               # h rows per src chunk
    if c["PACK4"]:
        c["CHN"] = c["N"] // 4                 # h4 table rows (4 nodes/row)
    assert c["IND"] or c["CHN"] <= 32767
    c["G"] = -(-c["S"] // c["SPG"])            # strip groups
    return c


def _plan(src, dst, w, cfg):
    """Bucket/sort/pad edges; build per-core device input streams."""
    c = cfg
    N, E, NC, NPC, S, CH, CHN = c["N"], c["E"], c["NC"], c["NPC"], c["S"], c["CH"], c["CHN"]

    src = np.asarray(src).astype(np.int64).ravel()
    dst = np.asarray(dst).astype(np.int64).ravel()
    w = np.asarray(w, dtype=np.float32).ravel()

    core = dst // NPC
    rem = dst - core * NPC
    strip = rem >> 7
    offs = rem & 127
    if c["IND"]:
        chunk = np.zeros_like(src)
        lsrc = src.astype(np.int32)
        quarter = None
    elif c["PACK4"]:
        chunk = np.zeros_like(src)
        lsrc = (src >> 2).astype(np.int16)
        quarter = (src & 3).astype(np.float32)
    else:
        chunk = src // CHN
        lsrc = (src - chunk * CHN).astype(np.int16)
        quarter = None

    cellkey = (core * S + strip) * CH + chunk
    counts = np.bincount(cellkey, minlength=NC * S * CH)
    NB = -(-counts.reshape(NC, S, CH).max(axis=0) // 128)  # [S, CH] blocks per cell

    # Tile order: (group, chunk, strip-in-group, block).
    cell_tile_start = np.zeros((S, CH), dtype=np.int64)
    t_acc = 0
    for g in range(c["G"]):
        s0, s1 = g * c["SPG"], min((g + 1) * c["SPG"], S)
        for ch in range(CH):
            for s in range(s0, s1):
                cell_tile_start[s, ch] = t_acc
                t_acc += NB[s, ch]
    T = int(t_acc)
    assert T == int(NB.sum())
    TE = T * 128

    # Scatter each edge to its position in its core's padded stream.
    order = np.lexsort((chunk, strip, core))
    core_s = core[order]
    starts = np.zeros(NC * S * CH + 1, dtype=np.int64)
    np.cumsum(counts, out=starts[1:])
    rank = np.arange(E, dtype=np.int64) - starts[cellkey[order]]
    pos = cell_tile_start[strip[order], chunk[order]] * 128 + rank

    pad_idx = -1 if int(os.environ.get("K_NEGPAD", "0")) else 0
    idx_stream = np.full((NC, TE), pad_idx,
                         dtype=np.int32 if c["IND"] else np.int16)
    offs_stream = np.full((NC, TE), -1.0, dtype=np.float32)
    w_stream = np.zeros((NC, TE), dtype=np.float32)
    idx_stream[core_s, pos] = lsrc[order]
    offs_stream[core_s, pos] = offs[order]
    w_stream[core_s, pos] = w[order]
    q_stream = None
    if c["PACK4"]:
        q_stream = np.zeros((NC, TE), dtype=np.float32)
        q_stream[core_s, pos] = quarter[order]
    if int(os.environ.get("K_ZIDX", "0")):  # perf probe: all gathers hit row 0
        idx_stream[:] = 0

    if c["IND"]:
        # int32 idx in the offs-style wrap: element e of tile t -> [e, t]
        idx_wrapped = np.ascontiguousarray(
            idx_stream.reshape(NC, T, 128).transpose(0, 2, 1)
        )
        offs_arr = np.ascontiguousarray(
            offs_stream.reshape(NC, T, 128).transpose(0, 2, 1)
        )
        w_arr = np.ascontiguousarray(w_stream.reshape(NC, T, 128).transpose(0, 2, 1))
        return NB, idx_wrapped, offs_arr, w_arr, None

    # idx: wrapped per (group, chunk) run: within-run element i -> [i%16, i//16],
    # replicated across the 8 GPSIMD core groups (128 partitions total).
    idx_wrapped = np.zeros((NC, 16, TE // 16), dtype=np.int16)
    run_t = 0
    for g in range(c["G"]):
        s0, s1 = g * c["SPG"], min((g + 1) * c["SPG"], S)
        for ch in range(CH):
            n = int(NB[s0:s1, ch].sum())
            if n == 0:
                continue
            seg = idx_stream[:, run_t * 128:(run_t + n) * 128]
            idx_wrapped[:, :, run_t * 8:(run_t + n) * 8] = (
                seg.reshape(NC, -1, 16).transpose(0, 2, 1)
            )
            run_t += n
    assert run_t == T

    # offs/w: wrapped globally per 128-edge tile: element i -> [i%128, i//128].
    offs_arr = np.ascontiguousarray(offs_stream.reshape(NC, T, 128).transpose(0, 2, 1))
    w_arr = np.ascontiguousarray(w_stream.reshape(NC, T, 128).transpose(0, 2, 1))
    q_arr = None
    if c["PACK4"]:
        q_arr = np.ascontiguousarray(q_stream.reshape(NC, T, 128).transpose(0, 2, 1))

    return NB, idx_wrapped, offs_arr, w_arr, q_arr


def _build(NB, cfg):
    """Build the Bass program (shared by all 8 cores)."""
    from concourse import bacc, tile, mybir

    c = cfg
    N, S, CH, CHN, G, SPG = c["N"], c["S"], c["CH"], c["CHN"], c["G"], c["SPG"]
    dt = mybir.dt
    T = int(NB.sum())

    # group chunk tile counts
    g_ncg = []
    for g in range(G):
        s0, s1 = g * SPG, min((g + 1) * SPG, S)
        g_ncg.append([int(NB[s0:s1, ch].sum()) for ch in range(CH)])
    NTG_MAX = max(sum(x) for x in g_ncg)

    fused = bool(int(os.environ.get("K_FUSED", "0")))
    repeat = int(os.environ.get("K_REPEAT", "1"))
    nq = int(os.environ.get("K_SWDGEQ", "4"))

    pack4 = c["PACK4"]
    ind = c["IND"]
    nc = bacc.Bacc(None, num_swdge_queues=nq)
    if ind:
        h_ext = nc.declare_dram_parameter("h", [N, 32], dt.bfloat16, isOutput=False)
        idx_ext = nc.declare_dram_parameter("idx", [128, T], dt.int32, isOutput=False)
    else:
        h_rows = c["CHN"] if pack4 else N
        h_ext = nc.declare_dram_parameter(
            "h", [h_rows, 128], dt.bfloat16, isOutput=False
        )
        idx_ext = nc.declare_dram_parameter(
            "idx", [128, T * 8], dt.int16, isOutput=False
        )
    offs_ext = nc.declare_dram_parameter("offs", [128, T], dt.float32, isOutput=False)
    wt_ext = nc.declare_dram_parameter("wt", [128, T], dt.float32, isOutput=False)
    q_ext = None
    if pack4:
        q_ext = nc.declare_dram_parameter("q", [128, T], dt.float32, isOutput=False)
    iota_ext = nc.declare_dram_parameter("iota", [128, 128], dt.bfloat16, isOutput=False)
    qiota_ext = None
    if pack4:
        qiota_ext = nc.declare_dram_parameter(
            "qiota", [128, 128], dt.bfloat16, isOutput=False
        )
    out_ext = nc.declare_dram_parameter("out", [S * 128, 32], dt.float32, isOutput=True)

    with tile.TileContext(nc) as tc:
        with (
            tc.tile_pool(name="const", bufs=1) as cpool,
            tc.tile_pool(name="gp", bufs=2) as gpool,
            tc.tile_pool(name="pwp", bufs=8 if (fused or pack4) else 2) as pwpool,
            tc.tile_pool(name="sm", bufs=2) as smpool,
            tc.tile_pool(name="outp", bufs=1) as opool,
            tc.tile_pool(name="ps", bufs=4, space="PSUM") as pspool,
        ):
            iota_t = cpool.tile([128, 128], dt.bfloat16)
            nc.sync.dma_start(out=iota_t[:], in_=iota_ext[:])
            qiota_t = None
            if pack4:
                qiota_t = cpool.tile([128, 128], dt.bfloat16)
                nc.sync.dma_start(out=qiota_t[:], in_=qiota_ext[:])
            out_sbuf = opool.tile([128, S * 32], dt.float32)

            def _emit_one_rep():
                if ind:
                    _emit_pipeline_ind(
                        nc, tile, mybir, dt, NB, c, g_ncg, NTG_MAX,
                        iota_t, out_sbuf,
                        gpool, pwpool, smpool, pspool,
                        h_ext, idx_ext, offs_ext, wt_ext,
                    )
                elif pack4:
                    _emit_pipeline_p4(
                        nc, tile, mybir, dt, NB, c, g_ncg, NTG_MAX,
                        iota_t, qiota_t, out_sbuf,
                        gpool, pwpool, smpool, pspool,
                        h_ext, idx_ext, offs_ext, wt_ext, q_ext,
                    )
                else:
                    _emit_pipeline(
                        nc, tile, mybir, dt, NB, c, g_ncg, NTG_MAX, fused,
                        iota_t, out_sbuf,
                        gpool, pwpool, smpool, pspool,
                        h_ext, idx_ext, offs_ext, wt_ext,
                    )

            use_loop = bool(int(os.environ.get("K_LOOP", "0")))
            if use_loop and repeat > 1:
                with tc.For_i(0, repeat) as _i:
                    _emit_one_rep()
            else:
                for _rep in range(repeat):
                    _emit_one_rep()

            nc.sync.dma_start(
                out=out_ext[:].rearrange("(s p) d -> p s d", p=128),
                in_=out_sbuf[:].rearrange("p (s d) -> p s d", d=32),
            )
    nc.finalize()
    return nc


def _emit_pipeline(
    nc, tile, mybir, dt, NB, c, g_ncg, NTG_MAX, fused,
    iota_t, out_sbuf,
    gpool, pwpool, smpool, pspool,
    h_ext, idx_ext, offs_ext, wt_ext,
):
    S, CH, CHN, G, SPG = c["S"], c["CH"], c["CHN"], c["G"], c["SPG"]
    qrr = bool(int(os.environ.get("K_QRR", "0")))
    qctr = [0]

    def next_q(nqs):
        q = qctr[0] % nqs
        qctr[0] += 1
        return q

    toff = 0
    for g in range(G):
        s0, s1 = g * SPG, min((g + 1) * SPG, S)
        ncg = g_ncg[g]
        ntg = sum(ncg)
        if ntg == 0:
            for s in range(s0, s1):
                nc.vector.memset(out_sbuf[:, s * 32:(s + 1) * 32], 0.0)
            continue

        gbuf = gpool.tile(
            [128, NTG_MAX * (64 if int(os.environ.get("K_HALF", "0")) else 128)],
            dt.bfloat16, tag="gbuf",
        )
        idx_t = smpool.tile([128, NTG_MAX * 8], dt.int16, tag="idx")
        offs_t = smpool.tile([128, NTG_MAX], dt.float32, tag="offs")
        wt_t = smpool.tile([128, NTG_MAX], dt.float32, tag="wt")

        nc.sync.dma_start(
            out=idx_t[:, : ntg * 8], in_=idx_ext[:, toff * 8:(toff + ntg) * 8]
        )
        nc.sync.dma_start(out=offs_t[:, :ntg], in_=offs_ext[:, toff:toff + ntg])
        nc.sync.dma_start(out=wt_t[:, :ntg], in_=wt_ext[:, toff:toff + ntg])

        half = bool(int(os.environ.get("K_HALF", "0")))
        ew = 64 if half else 128  # gathered elem width (bf16 elems)
        subg = int(os.environ.get("K_SUBG", "0"))  # tiles per sub-gather (0=off)
        nqs = max(1, int(os.environ.get("K_SWDGEQ", "4")))
        skip_gather = bool(int(os.environ.get("K_SKIP_GATHER", "0")))
        skip_compute = bool(int(os.environ.get("K_SKIP_COMPUTE", "0")))
        g3 = gbuf[:].rearrange("p (t e) -> p t e", e=ew)
        co = 0
        for ch in range(CH):
            n = ncg[ch]
            if n == 0 or skip_gather:
                continue
            step = subg if subg else n
            for o in range(0, n, step):
                m = min(step, n - o)
                nc.gpsimd.dma_gather(
                    out_ap=g3[:, co + o:co + o + m, :],
                    in_ap=h_ext[ch * CHN:(ch + 1) * CHN, :ew],
                    idxs_ap=idx_t[:, (co + o) * 8:(co + o + m) * 8],
                    num_idxs=m * 128,
                    num_idxs_reg=m * 128,
                    elem_size=ew,
                    elem_step=128,
                    # single-packet desc-gen faults above 1024 idxs
                    single_packet=(m * 128 <= 1024) if subg else False,
                    queue_num=next_q(nqs) if qrr else ch % nqs,
                )
            co += n

        if skip_compute:
            for s in range(s0, s1):
                nc.vector.memset(out_sbuf[:, s * 32:(s + 1) * 32], 0.0)
            toff += ntg
            continue

        if not fused:
            pw = pwpool.tile([128, NTG_MAX * 128], dt.bfloat16, tag="pw")
            pw3 = pw[:].rearrange("p (t e) -> p t e", e=128)
            # Broadcast per-edge dst offsets across the 128 one-hot columns.
            nc.scalar.activation(
                out=pw3[:, :ntg, :],
                in_=offs_t[:, :ntg].unsqueeze(2).broadcast_to([128, ntg, 128]),
                func=mybir.ActivationFunctionType.Copy,
            )
            # One-hot: P[e, j] = (offs_e == j)
            nc.vector.tensor_tensor(
                out=pw3[:, :ntg, :],
                in0=iota_t[:].unsqueeze(1).broadcast_to([128, ntg, 128]),
                in1=pw3[:, :ntg, :],
                op=mybir.AluOpType.is_equal,
            )
            # msg = h[src] * w (in place on the used 32 columns)
            nc.vector.tensor_tensor(
                out=g3[:, :ntg, 0:32],
                in0=g3[:, :ntg, 0:32],
                in1=wt_t[:, :ntg].unsqueeze(2).broadcast_to([128, ntg, 32]),
                op=mybir.AluOpType.mult,
            )

        chunk_base = np.concatenate([[0], np.cumsum(ncg)]).astype(int)
        for s in range(s0, s1):
            nb = int(NB[s].sum())
            if nb == 0:
                nc.vector.memset(out_sbuf[:, s * 32:(s + 1) * 32], 0.0)
                continue
            ps = pspool.tile([128, 32], dt.float32)
            bi = 0
            for ch in range(CH):
                nbs = int(NB[s, ch])
                if nbs == 0:
                    continue
                lt0 = int(chunk_base[ch] + NB[s0:s, ch].sum())
                for b in range(nbs):
                    t = lt0 + b
                    if fused:
                        # P_w[e, j] = (offs_e == j) * w_e in one DVE op
                        pwb = pwpool.tile([128, 128], dt.bfloat16, tag="pwb")
                        nc.vector.tensor_scalar(
                            out=pwb[:],
                            in0=iota_t[:],
                            scalar1=offs_t[:, t:t + 1],
                            scalar2=wt_t[:, t:t + 1],
                            op0=mybir.AluOpType.is_equal,
                            op1=mybir.AluOpType.mult,
                        )
                        lhs = pwb[:]
                    else:
                        lhs = pw[:, t * 128:(t + 1) * 128]
                    nc.tensor.matmul(
                        out=ps[:],
                        lhsT=lhs,
                        rhs=g3[:, t, 0:32],
                        start=(bi == 0),
                        stop=(bi == nb - 1),
                    )
                    bi += 1
            nc.scalar.copy(out=out_sbuf[:, s * 32:(s + 1) * 32], in_=ps[:])
        toff += ntg


def _emit_pipeline_p4(
    nc, tile, mybir, dt, NB, c, g_ncg, NTG_MAX,
    iota_t, qiota_t, out_sbuf,
    gpool, pwpool, smpool, pspool,
    h_ext, idx_ext, offs_ext, wt_ext, q_ext,
):
    """Pack-4 pipeline: h4 table [N/4, 128] bf16 holds 4 nodes per 256B row.

    Per tile: fused one-hot P=(iota==offs)*w (DVE), quarter-select
    g3=(qiota==q)*g3 (DVE stt), matmul -> psum [v,128]=4 quarter-partials,
    per-strip fold via strided tensor_reduce.
    """
    S, CHN, G, SPG = c["S"], c["CHN"], c["G"], c["SPG"]
    subg = int(os.environ.get("K_SUBG", "0"))  # tiles per sub-gather (0=off)
    nqs = max(1, int(os.environ.get("K_SWDGEQ", "4")))
    skip_gather = bool(int(os.environ.get("K_SKIP_GATHER", "0")))
    skip_compute = bool(int(os.environ.get("K_SKIP_COMPUTE", "0")))
    qrr = bool(int(os.environ.get("K_QRR", "1")))
    qctr = [0]

    def next_q():
        q = qctr[0] % nqs
        qctr[0] += 1
        return q

    toff = 0
    for g in range(G):
        s0, s1 = g * SPG, min((g + 1) * SPG, S)
        ntg = g_ncg[g][0]
        if ntg == 0:
            for s in range(s0, s1):
                nc.vector.memset(out_sbuf[:, s * 32:(s + 1) * 32], 0.0)
            continue

        gbuf = gpool.tile([128, NTG_MAX * 128], dt.bfloat16, tag="gbuf")
        idx_t = smpool.tile([128, NTG_MAX * 8], dt.int16, tag="idx")
        offs_t = smpool.tile([128, NTG_MAX], dt.float32, tag="offs")
        wt_t = smpool.tile([128, NTG_MAX], dt.float32, tag="wt")
        q_t = smpool.tile([128, NTG_MAX], dt.float32, tag="q")

        nc.sync.dma_start(
            out=idx_t[:, : ntg * 8], in_=idx_ext[:, toff * 8:(toff + ntg) * 8]
        )
        nc.sync.dma_start(out=offs_t[:, :ntg], in_=offs_ext[:, toff:toff + ntg])
        nc.sync.dma_start(out=wt_t[:, :ntg], in_=wt_ext[:, toff:toff + ntg])
        nc.sync.dma_start(out=q_t[:, :ntg], in_=q_ext[:, toff:toff + ntg])

        g3 = gbuf[:].rearrange("p (t e) -> p t e", e=128)
        if not skip_gather:
            step = subg if subg else ntg
            for o in range(0, ntg, step):
                m = min(step, ntg - o)
                nc.gpsimd.dma_gather(
                    out_ap=g3[:, o:o + m, :],
                    in_ap=h_ext[0:CHN, :],
                    idxs_ap=idx_t[:, o * 8:(o + m) * 8],
                    num_idxs=m * 128,
                    num_idxs_reg=m * 128,
                    elem_size=128,
                    elem_step=128,
                    single_packet=(m * 128 <= 1024) if subg else False,
                    queue_num=next_q() if qrr else 0,
                )

        if skip_compute:
            for s in range(s0, s1):
                nc.vector.memset(out_sbuf[:, s * 32:(s + 1) * 32], 0.0)
            toff += ntg
            continue

        for s in range(s0, s1):
            nb = int(NB[s, 0])
            if nb == 0:
                nc.vector.memset(out_sbuf[:, s * 32:(s + 1) * 32], 0.0)
                continue
            ps = pspool.tile([128, 128], dt.float32)
            lt0 = int(NB[s0:s, 0].sum())
            for b in range(nb):
                t = lt0 + b
                # P[e, v] = (iota == offs_e) * w_e  (one DVE op, bf16)
                pwb = pwpool.tile([128, 128], dt.bfloat16, tag="pwb")
                nc.vector.tensor_scalar(
                    out=pwb[:],
                    in0=iota_t[:],
                    scalar1=offs_t[:, t:t + 1],
                    scalar2=wt_t[:, t:t + 1],
                    op0=mybir.AluOpType.is_equal,
                    op1=mybir.AluOpType.mult,
                )
                # quarter-select in place: g3 = (qiota == q_e) * g3
                nc.vector.scalar_tensor_tensor(
                    out=g3[:, t, :],
                    in0=qiota_t[:],
                    scalar=q_t[:, t:t + 1],
                    in1=g3[:, t, :],
                    op0=mybir.AluOpType.is_equal,
                    op1=mybir.AluOpType.mult,
                )
                nc.tensor.matmul(
                    out=ps[:],
                    lhsT=pwb[:],
                    rhs=g3[:, t, :],
                    start=(b == 0),
                    stop=(b == nb - 1),
                )
            # fold the 4 quarter partials: out[v, f] = sum_q ps[v, 32q+f]
            nc.vector.tensor_reduce(
                out=out_sbuf[:, s * 32:(s + 1) * 32],
                in_=ps[:].rearrange("p (q f) -> p f q", f=32),
                axis=mybir.AxisListType.X,
                op=mybir.AluOpType.add,
            )
        toff += ntg


def _emit_pipeline_ind(
    nc, tile, mybir, dt, NB, c, g_ncg, NTG_MAX,
    iota_t, out_sbuf,
    gpool, pwpool, smpool, pspool,
    h_ext, idx_ext, offs_ext, wt_ext,
):
    """Indirect-DMA pipeline: per-edge 64B rows h[src] gathered via the
    dynamic-DGE path (int32 offsets, one desc per edge, 16 DMA engines).

    Per tile: fused one-hot P=(iota==offs)*w (DVE), matmul [K=128e, M=128v,
    N=32f] accumulating per-strip PSUM, per-strip copy to out_sbuf.
    """
    from concourse import bass

    S, G, SPG = c["S"], c["G"], c["SPG"]
    subg = int(os.environ.get("K_SUBG", "0"))  # tiles per sub-gather (0=off)
    skip_gather = bool(int(os.environ.get("K_SKIP_GATHER", "0")))
    skip_compute = bool(int(os.environ.get("K_SKIP_COMPUTE", "0")))

    toff = 0
    for g in range(G):
        s0, s1 = g * SPG, min((g + 1) * SPG, S)
        ntg = g_ncg[g][0]
        if ntg == 0:
            for s in range(s0, s1):
                nc.vector.memset(out_sbuf[:, s * 32:(s + 1) * 32], 0.0)
            continue

        gbuf = gpool.tile([128, NTG_MAX * 32], dt.bfloat16, tag="gbuf")
        idx_t = smpool.tile([128, NTG_MAX], dt.int32, tag="idx")
        offs_t = smpool.tile([128, NTG_MAX], dt.float32, tag="offs")
        wt_t = smpool.tile([128, NTG_MAX], dt.float32, tag="wt")

        nc.sync.dma_start(out=idx_t[:, :ntg], in_=idx_ext[:, toff:toff + ntg])
        nc.sync.dma_start(out=offs_t[:, :ntg], in_=offs_ext[:, toff:toff + ntg])
        nc.sync.dma_start(out=wt_t[:, :ntg], in_=wt_ext[:, toff:toff + ntg])

        g3 = gbuf[:].rearrange("p (t e) -> p t e", e=32)
        if not skip_gather:
            step = subg if subg else ntg
            for o in range(0, ntg, step):
                m = min(step, ntg - o)
                nc.gpsimd.indirect_dma_start(
                    out=g3[:, o:o + m, :],
                    out_offset=None,
                    in_=h_ext[:],
                    in_offset=bass.IndirectOffsetOnAxis(
                        ap=idx_t[:, o:o + m], axis=0
                    ),
                )

        if skip_compute:
            for s in range(s0, s1):
                nc.vector.memset(out_sbuf[:, s * 32:(s + 1) * 32], 0.0)
            toff += ntg
            continue

        for s in range(s0, s1):
            nb = int(NB[s, 0])
            if nb == 0:
                nc.vector.memset(out_sbuf[:, s * 32:(s + 1) * 32], 0.0)
                continue
            ps = pspool.tile([128, 32], dt.float32)
            lt0 = int(NB[s0:s, 0].sum())
            for b in range(nb):
                t = lt0 + b
                # P[e, v] = (iota == offs_e) * w_e  (one DVE op, bf16)
                pwb = pwpool.tile([128, 128], dt.bfloat16, tag="pwb")
                nc.vector.tensor_scalar(
                    out=pwb[:],
                    in0=iota_t[:],
                    scalar1=offs_t[:, t:t + 1],
                    scalar2=wt_t[:, t:t + 1],
                    op0=mybir.AluOpType.is_equal,
                    op1=mybir.AluOpType.mult,
                )
                nc.tensor.matmul(
                    out=ps[:],
                    lhsT=pwb[:],
                    rhs=g3[:, t, :],
                    start=(b == 0),
                    stop=(b == nb - 1),
                )
            nc.scalar.copy(out=out_sbuf[:, s * 32:(s + 1) * 32], in_=ps[:])
        toff += ntg


def _make_in_maps(h, c, NB, idx_wrapped, offs_arr, w_arr, q_arr):
    N, D, NC = c["N"], c["D"], c["NC"]
    iota = np.broadcast_to(
        np.arange(128, dtype=np.float32).astype(BF16), (128, 128)
    ).copy()
    if c["IND"]:
        h_bf = np.ascontiguousarray(np.asarray(h, dtype=np.float32).astype(BF16))
        return [
            {
                "h": h_bf,
                "idx": idx_wrapped[i],
                "offs": offs_arr[i],
                "wt": w_arr[i],
                "iota": iota,
            }
            for i in range(NC)
        ]
    if c["PACK4"]:
        h4 = np.asarray(h, dtype=np.float32).astype(BF16)
        assert h4.shape == (N, D) and D * 4 == 128
        h4 = np.ascontiguousarray(h4.reshape(N // 4, 128))
        qiota = np.broadcast_to(
            (np.arange(128) // 32).astype(np.float32).astype(BF16), (128, 128)
        ).copy()
        return [
            {
                "h": h4,
                "idx": np.ascontiguousarray(np.tile(idx_wrapped[i], (8, 1))),
                "offs": offs_arr[i],
                "wt": w_arr[i],
                "q": q_arr[i],
                "iota": iota,
                "qiota": qiota,
            }
            for i in range(NC)
        ]
    h_pad = np.zeros((N, 128), dtype=BF16)
    h_pad[:, :D] = np.asarray(h, dtype=np.float32).astype(BF16)
    return [
        {
            "h": h_pad,
            "idx": np.ascontiguousarray(np.tile(idx_wrapped[i], (8, 1))),
            "offs": offs_arr[i],
            "wt": w_arr[i],
            "iota": iota,
        }
        for i in range(NC)
    ]


def run_cfg(h, w, src, dst, cfg, trace=False):
    from concourse.bass_utils import run_bass_kernel_spmd

    c = _derive(cfg)
    N, D, NC, NPC, S = c["N"], c["D"], c["NC"], c["NPC"], c["S"]

    NB, idx_wrapped, offs_arr, w_arr, q_arr = _plan(src, dst, w, c)
    nc = _build(NB, c)

    in_maps = _make_in_maps(h, c, NB, idx_wrapped, offs_arr, w_arr, q_arr)
    res = run_bass_kernel_spmd(nc, in_maps, list(range(NC)), trace=trace)
    out = np.empty((N, D), dtype=np.float32)
    for i in range(NC):
        out[i * NPC:(i + 1) * NPC] = res.results[i]["out"][:NPC]
    return out, res


def make_runner(h, w, src, dst, cfg):
    """Build a reusable jitted SPMD callable for timing: returns
    (run_once, assemble) where run_once() returns unblocked device arrays."""
    import jax
    import jax.numpy as jnp
    from jax.sharding import Mesh, PartitionSpec, NamedSharding
    from jax.experimental.shard_map import shard_map
    from concourse import bass2jax, mybir

    c = _derive(cfg)
    N, D, NC, NPC = c["N"], c["D"], c["NC"], c["NPC"]

    NB, idx_wrapped, offs_arr, w_arr, q_arr = _plan(src, dst, w, c)
    nc = _build(NB, c)

    in_maps = _make_in_maps(h, c, NB, idx_wrapped, offs_arr, w_arr, q_arr)

    bass2jax.install_neuronx_cc_hook()
    partition_name = nc.partition_id_tensor.name if nc.partition_id_tensor else None
    in_names, out_names, out_avals, zero_shapes = [], [], [], []
    for alloc in nc.m.functions[0].allocations:
        if not isinstance(alloc, mybir.MemoryLocationSet):
            continue
        name = alloc.memorylocations[0].name
        if alloc.kind == "ExternalInput":
            if name != partition_name:
                in_names.append(name)
        elif alloc.kind == "ExternalOutput":
            out_names.append(name)
            shape = tuple(alloc.tensor_shape)
            dtype = mybir.dt.np(alloc.dtype)
            out_avals.append(jax.core.ShapedArray(shape, dtype))
            zero_shapes.append((shape, dtype))
    n_params = len(in_names)
    n_outs = len(out_avals)
    all_in_names = list(in_names) + list(out_names)
    if partition_name is not None:
        all_in_names.append(partition_name)

    def _body(*args):
        operands = list(args)
        if partition_name is not None:
            operands.append(bass2jax.partition_id_tensor())
        outs = bass2jax._bass_exec_p.bind(
            *operands,
            out_avals=tuple(out_avals),
            in_names=tuple(all_in_names),
            out_names=tuple(out_names),
            lowering_input_output_aliases=(),
            sim_require_finite=True,
            sim_require_nnan=True,
            nc=nc,
        )
        return tuple(outs)

    devices = jax.devices()[:NC]
    mesh = Mesh(np.asarray(devices), ("core",))
    donate = tuple(range(n_params, n_params + n_outs))
    sharded = jax.jit(
        shard_map(
            _body,
            mesh=mesh,
            in_specs=(PartitionSpec("core"),) * (n_params + n_outs),
            out_specs=(PartitionSpec("core"),) * n_outs,
            check_rep=False,
        ),
        donate_argnums=donate,
        keep_unused=True,
    )

    concat_in = [
        np.concatenate([np.asarray(in_maps[k][nm]) for k in range(NC)], axis=0)
        for nm in in_names
    ]
    shard = NamedSharding(mesh, PartitionSpec("core"))
    dev_in = [jax.device_put(a, shard) for a in concat_in]

    zeros_fn = jax.jit(
        lambda: tuple(
            jnp.zeros((NC * s[0], *s[1:]), dt) for (s, dt) in zero_shapes
        ),
        out_shardings=(shard,) * n_outs,
    )

    def run_once():
        zs = zeros_fn()
        return sharded(*dev_in, *zs)

    def assemble(out_arrs):
        full = np.empty((N, D), dtype=np.float32)
        o = np.asarray(out_arrs[0]).reshape(NC, -1, D)
        for i in range(NC):
            full[i * NPC:(i + 1) * NPC] = o[i, :NPC]
        return full

    # chained executor: K back-to-back executions in ONE dispatch, each
    # feeding its output as the next call's out-operand (defeats CSE).
    def make_chain(k):
        def _chain_body(*args):
            ins, outs = args[:n_params], list(args[n_params:])
            for _ in range(k):
                outs = list(_body(*ins, *outs))
            return tuple(outs)

        return jax.jit(
            shard_map(
                _chain_body,
                mesh=mesh,
                in_specs=(PartitionSpec("core"),) * (n_params + n_outs),
                out_specs=(PartitionSpec("core"),) * n_outs,
                check_rep=False,
            ),
            donate_argnums=donate,
            keep_unused=True,
        )

    def run_chain(chain_fn):
        zs = zeros_fn()
        return chain_fn(*dev_in, *zs)

    return run_once, assemble, make_chain, run_chain


def kernel(**inputs):
    out, _ = run_cfg(
        inputs["h"], inputs["w"], inputs["src"], inputs["dst"], FULL_CFG
    )
    return out



# revision 32
# speedup vs baseline: 1.2221x; 1.2221x over previous
"""GNN message passing (u_mul_e -> segment_sum) on 8 Trainium2 NeuronCores.

out[v] = sum_{e=(u->v)} h[u] * w[e]

Strategy (edge/graph parallelism, dst-range sharded -> no collectives):
  - Each core owns a contiguous range of 12500 destination nodes.
  - Host buckets edges by (core, 128-node dst strip, src chunk), sorts, and
    pads each bucket to a multiple of 128 edges, uniformly across cores
    (same instruction stream on every core = SPMD).
  - Device, per group of SPG=5 strips (20 pipeline groups, double-buffered):
      * dma_gather pulls h rows (bf16, padded to 256B) for each edge's src —
        one 256B descriptor per edge, one un-split gather per (group, chunk),
        spread over all 4 SWDGE queues (queue = chunk % 4). The gather is the
        bottleneck: measured ~3.0 ns/descriptor at 4 queues (descriptor-count
        proportional; random src is FASTER than repeated rows since it
        spreads across HBM banks).
      * DVE builds a weighted one-hot matrix P[e, j] = (offs_e == j) per
        128-edge block in bulk (is_equal against a broadcast iota, Act does
        the offs broadcast), and multiplies gathered h rows by w. Bulk ops
        keep per-tile dependency waits off the DVE sequencer so compute
        hides entirely under the gather.
      * PE computes P^T @ msg per strip, accumulating the [128, 32] segment
        sums in PSUM across the strip's blocks.
  - Per-strip results are copied to an SBUF output buffer and DMA'd out once.

Perf knobs (env, defaults tuned): K_SPG=5, K_SWDGEQ=4, K_SUBG=0.
Measured: ~421 us/exec steady-state (median interleaved-pair slope, 21-rep
device-unrolled), ~600 us/exec with a full barrier between reps (For_i loop
mode, K_LOOP=1); baseline was 841 us. Alternative gather modes kept for
reference: K_PACK4 (4 nodes per 256B row, 15% fewer descriptors but its
per-tile DVE chain exposes ~600 us), K_GMODE=ind (indirect/dynamic-DGE
64B-row gather — faster on paper but overflows the 1024-descriptor dynamic
ring and desyncs the device; do not use).
"""

import os
import sys

sys.path.insert(0, "/opt/trn_rl_repo")

import numpy as np
import ml_dtypes

BF16 = ml_dtypes.bfloat16

# Full-problem configuration (hardcoded; kernel.py must be self-contained).
# Tuned: SPG=5 (smaller pipeline groups overlap gather/compute best) and
# 4 SWDGE queues with one un-split gather per (group, chunk) run.
FULL_CFG = dict(
    N=100000,   # nodes
    E=1600000,  # edges
    D=32,       # feature dim
    NC=8,       # cores
    CH=4,       # src chunks (N/CH must be <= 32767 for int16 gather indices)
    SPG=int(os.environ.get("K_SPG", "5")),  # dst strips (128 nodes each) per pipeline group
)


def _derive(cfg):
    c = dict(cfg)
    c["PACK4"] = bool(int(os.environ.get("K_PACK4", "0")))
    c["IND"] = os.environ.get("K_GMODE", "swdge") == "ind"
    if c["IND"]:
        c["PACK4"] = False
        c["CH"] = 1
    if c["PACK4"]:
        c["CH"] = 1
    assert c["N"] % c["NC"] == 0
    c["NPC"] = c["N"] // c["NC"]               # dst nodes per core
    c["S"] = -(-c["NPC"] // 128)               # strips per core
    assert c["N"] % c["CH"] == 0
    c["CHN"] = c["N"] // c["CH"]               # h rows per src chunk
    if c["PACK4"]:
        c["CHN"] = c["N"] // 4                 # h4 table rows (4 nodes/row)
    assert c["IND"] or c["CHN"] <= 32767
    c["G"] = -(-c["S"] // c["SPG"])            # strip groups
    return c


def _plan(src, dst, w, cfg):
    """Bucket/sort/pad edges; build per-core device input streams."""
    c = cfg
    N, E, NC, NPC, S, CH, CHN = c["N"], c["E"], c["NC"], c["NPC"], c["S"], c["CH"], c["CHN"]

    src = np.asarray(src).astype(np.int64).ravel()
    dst = np.asarray(dst).astype(np.int64).ravel()
    w = np.asarray(w, dtype=np.float32).ravel()

    core = dst // NPC
    rem = dst - core * NPC
    strip = rem >> 7
    offs = rem & 127
    if c["IND"]:
        chunk = np.zeros_like(src)
        lsrc = src.astype(np.int32)
        quarter = None
    elif c["PACK4"]:
        chunk = np.zeros_like(src)
        lsrc = (src >> 2).astype(np.int16)
        quarter = (src & 3).astype(np.float32)
    else:
        chunk = src // CHN
        lsrc = (src - chunk * CHN).astype(np.int16)
        quarter = None

    cellkey = (core * S + strip) * CH + chunk
    counts = np.bincount(cellkey, minlength=NC * S * CH)
    NB = -(-counts.reshape(NC, S, CH).max(axis=0) // 128)  # [S, CH] blocks per cell

    # Tile order: (group, chunk, strip-in-group, block).
    cell_tile_start = np.zeros((S, CH), dtype=np.int64)
    t_acc = 0
    for g in range(c["G"]):
        s0, s1 = g * c["SPG"], min((g + 1) * c["SPG"], S)
        for ch in range(CH):
            for s in range(s0, s1):
                cell_tile_start[s, ch] = t_acc
                t_acc += NB[s, ch]
    T = int(t_acc)
    assert T == int(NB.sum())
    TE = T * 128

    # Scatter each edge to its position in its core's padded stream.
    order = np.lexsort((chunk, strip, core))
    core_s = core[order]
    starts = np.zeros(NC * S * CH + 1, dtype=np.int64)
    np.cumsum(counts, out=starts[1:])
    rank = np.arange(E, dtype=np.int64) - starts[cellkey[order]]
    pos = cell_tile_start[strip[order], chunk[order]] * 128 + rank

    pad_idx = -1 if int(os.environ.get("K_NEGPAD", "0")) else 0
    idx_stream = np.full((NC, TE), pad_idx,
                         dtype=np.int32 if c["IND"] else np.int16)
    offs_stream = np.full((NC, TE), -1.0, dtype=np.float32)
    w_stream = np.zeros((NC, TE), dtype=np.float32)
    idx_stream[core_s, pos] = lsrc[order]
    offs_stream[core_s, pos] = offs[order]
    w_stream[core_s, pos] = w[order]
    q_stream = None
    if c["PACK4"]:
        q_stream = np.zeros((NC, TE), dtype=np.float32)
        q_stream[core_s, pos] = quarter[order]
    if int(os.environ.get("K_ZIDX", "0")):  # perf probe: all gathers hit row 0
        idx_stream[:] = 0

    if c["IND"]:
        # int32 idx in the offs-style wrap: element e of tile t -> [e, t]
        idx_wrapped = np.ascontiguousarray(
            idx_stream.reshape(NC, T, 128).transpose(0, 2, 1)
        )
        offs_arr = np.ascontiguousarray(
            offs_stream.reshape(NC, T, 128).transpose(0, 2, 1)
        )
        w_arr = np.ascontiguousarray(w_stream.reshape(NC, T, 128).transpose(0, 2, 1))
        return NB, idx_wrapped, offs_arr, w_arr, None

    # idx: wrapped per (group, chunk) run: within-run element i -> [i%16, i//16],
    # replicated across the 8 GPSIMD core groups (128 partitions total).
    idx_wrapped = np.zeros((NC, 16, TE // 16), dtype=np.int16)
    run_t = 0
    for g in range(c["G"]):
        s0, s1 = g * c["SPG"], min((g + 1) * c["SPG"], S)
        for ch in range(CH):
            n = int(NB[s0:s1, ch].sum())
            if n == 0:
                continue
            seg = idx_stream[:, run_t * 128:(run_t + n) * 128]
            idx_wrapped[:, :, run_t * 8:(run_t + n) * 8] = (
                seg.reshape(NC, -1, 16).transpose(0, 2, 1)
            )
            run_t += n
    assert run_t == T

    # offs/w: wrapped globally per 128-edge tile: element i -> [i%128, i//128].
    offs_arr = np.ascontiguousarray(offs_stream.reshape(NC, T, 128).transpose(0, 2, 1))
    w_arr = np.ascontiguousarray(w_stream.reshape(NC, T, 128).transpose(0, 2, 1))
    q_arr = None
    if c["PACK4"]:
        q_arr = np.ascontiguousarray(q_stream.reshape(NC, T, 128).transpose(0, 2, 1))

    return NB, idx_wrapped, offs_arr, w_arr, q_arr


def _build(NB, cfg):
    """Build the Bass program (shared by all 8 cores)."""
    from concourse import bacc, tile, mybir

    c = cfg
    N, S, CH, CHN, G, SPG = c["N"], c["S"], c["CH"], c["CHN"], c["G"], c["SPG"]
    dt = mybir.dt
    T = int(NB.sum())

    # group chunk tile counts
    g_ncg = []
    for g in range(G):
        s0, s1 = g * SPG, min((g + 1) * SPG, S)
        g_ncg.append([int(NB[s0:s1, ch].sum()) for ch in range(CH)])
    NTG_MAX = max(sum(x) for x in g_ncg)

    fused = bool(int(os.environ.get("K_FUSED", "0")))
    repeat = int(os.environ.get("K_REPEAT", "1"))
    nq = int(os.environ.get("K_SWDGEQ", "4"))

    pack4 = c["PACK4"]
    ind = c["IND"]
    nc = bacc.Bacc(None, num_swdge_queues=nq)
    if ind:
        h_ext = nc.declare_dram_parameter("h", [N, 32], dt.bfloat16, isOutput=False)
        idx_ext = nc.declare_dram_parameter("idx", [128, T], dt.int32, isOutput=False)
    else:
        h_rows = c["CHN"] if pack4 else N
        h_ext = nc.declare_dram_parameter(
            "h", [h_rows, 128], dt.bfloat16, isOutput=False
        )
        idx_ext = nc.declare_dram_parameter(
            "idx", [128, T * 8], dt.int16, isOutput=False
        )
    offs_ext = nc.declare_dram_parameter("offs", [128, T], dt.float32, isOutput=False)
    wt_ext = nc.declare_dram_parameter("wt", [128, T], dt.float32, isOutput=False)
    q_ext = None
    if pack4:
        q_ext = nc.declare_dram_parameter("q", [128, T], dt.float32, isOutput=False)
    iota_ext = nc.declare_dram_parameter("iota", [128, 128], dt.bfloat16, isOutput=False)
    qiota_ext = None
    if pack4:
        qiota_ext = nc.declare_dram_parameter(
            "qiota", [128, 128], dt.bfloat16, isOutput=False
        )
    out_ext = nc.declare_dram_parameter("out", [S * 128, 32], dt.float32, isOutput=True)

    # Triple-buffer the gather/stream pools: a third group's gathers enter
    # flight at group transitions, keeping all 4 SWDGE queues occupied
    # (measured ~15% faster than double buffering).
    gbufs = int(os.environ.get("K_GBUFS", "3"))
    with tile.TileContext(nc) as tc:
        with (
            tc.tile_pool(name="const", bufs=1) as cpool,
            tc.tile_pool(name="gp", bufs=gbufs) as gpool,
            tc.tile_pool(name="pwp", bufs=8 if fused else 2) as pwpool,
            tc.tile_pool(name="sm", bufs=gbufs) as smpool,
            tc.tile_pool(name="outp", bufs=1) as opool,
            tc.tile_pool(name="ps", bufs=4, space="PSUM") as pspool,
        ):
            iota_t = cpool.tile([128, 128], dt.bfloat16)
            nc.sync.dma_start(out=iota_t[:], in_=iota_ext[:])
            qiota_t = None
            if pack4:
                qiota_t = cpool.tile([128, 128], dt.bfloat16)
                nc.sync.dma_start(out=qiota_t[:], in_=qiota_ext[:])
            out_sbuf = opool.tile([128, S * 32], dt.float32)

            def _emit_one_rep():
                if ind:
                    _emit_pipeline_ind(
                        nc, tile, mybir, dt, NB, c, g_ncg, NTG_MAX,
                        iota_t, out_sbuf,
                        gpool, pwpool, smpool, pspool,
                        h_ext, idx_ext, offs_ext, wt_ext,
                    )
                elif pack4:
                    _emit_pipeline_p4(
                        nc, tile, mybir, dt, NB, c, g_ncg, NTG_MAX,
                        iota_t, qiota_t, out_sbuf,
                        gpool, pwpool, smpool, pspool,
                        h_ext, idx_ext, offs_ext, wt_ext, q_ext,
                    )
                else:
                    _emit_pipeline(
                        nc, tile, mybir, dt, NB, c, g_ncg, NTG_MAX, fused,
                        iota_t, out_sbuf,
                        gpool, pwpool, smpool, pspool,
                        h_ext, idx_ext, offs_ext, wt_ext,
                    )

            use_loop = bool(int(os.environ.get("K_LOOP", "0")))
            if use_loop and repeat > 1:
                with tc.For_i(0, repeat) as _i:
                    _emit_one_rep()
            else:
                for _rep in range(repeat):
                    _emit_one_rep()

            nc.sync.dma_start(
                out=out_ext[:].rearrange("(s p) d -> p s d", p=128),
                in_=out_sbuf[:].rearrange("p (s d) -> p s d", d=32),
            )
    nc.finalize()
    return nc


def _emit_pipeline(
    nc, tile, mybir, dt, NB, c, g_ncg, NTG_MAX, fused,
    iota_t, out_sbuf,
    gpool, pwpool, smpool, pspool,
    h_ext, idx_ext, offs_ext, wt_ext,
):
    S, CH, CHN, G, SPG = c["S"], c["CH"], c["CHN"], c["G"], c["SPG"]
    qrr = bool(int(os.environ.get("K_QRR", "0")))
    qctr = [0]

    def next_q(nqs):
        q = qctr[0] % nqs
        qctr[0] += 1
        return q

    toff = 0
    for g in range(G):
        s0, s1 = g * SPG, min((g + 1) * SPG, S)
        ncg = g_ncg[g]
        ntg = sum(ncg)
        if ntg == 0:
            for s in range(s0, s1):
                nc.vector.memset(out_sbuf[:, s * 32:(s + 1) * 32], 0.0)
            continue

        gbuf = gpool.tile(
            [128, NTG_MAX * (64 if int(os.environ.get("K_HALF", "0")) else 128)],
            dt.bfloat16, tag="gbuf",
        )
        idx_t = smpool.tile([128, NTG_MAX * 8], dt.int16, tag="idx")
        offs_t = smpool.tile([128, NTG_MAX], dt.float32, tag="offs")
        wt_t = smpool.tile([128, NTG_MAX], dt.float32, tag="wt")

        nc.sync.dma_start(
            out=idx_t[:, : ntg * 8], in_=idx_ext[:, toff * 8:(toff + ntg) * 8]
        )
        nc.sync.dma_start(out=offs_t[:, :ntg], in_=offs_ext[:, toff:toff + ntg])
        nc.sync.dma_start(out=wt_t[:, :ntg], in_=wt_ext[:, toff:toff + ntg])

        half = bool(int(os.environ.get("K_HALF", "0")))
        ew = 64 if half else 128  # gathered elem width (bf16 elems)
        subg = int(os.environ.get("K_SUBG", "0"))  # tiles per sub-gather (0=off)
        nqs = max(1, int(os.environ.get("K_SWDGEQ", "4")))
        skip_gather = bool(int(os.environ.get("K_SKIP_GATHER", "0")))
        skip_compute = bool(int(os.environ.get("K_SKIP_COMPUTE", "0")))
        g3 = gbuf[:].rearrange("p (t e) -> p t e", e=ew)
        co = 0
        for ch in range(CH):
            n = ncg[ch]
            if n == 0 or skip_gather:
                continue
            step = subg if subg else n
            for o in range(0, n, step):
                m = min(step, n - o)
                nc.gpsimd.dma_gather(
                    out_ap=g3[:, co + o:co + o + m, :],
                    in_ap=h_ext[ch * CHN:(ch + 1) * CHN, :ew],
                    idxs_ap=idx_t[:, (co + o) * 8:(co + o + m) * 8],
                    num_idxs=m * 128,
                    num_idxs_reg=m * 128,
                    elem_size=ew,
                    elem_step=128,
                    # single-packet desc-gen faults above 1024 idxs
                    single_packet=(m * 128 <= 1024) if subg else False,
                    queue_num=next_q(nqs) if qrr else ch % nqs,
                )
            co += n

        if skip_compute:
            for s in range(s0, s1):
                nc.vector.memset(out_sbuf[:, s * 32:(s + 1) * 32], 0.0)
            toff += ntg
            continue

        if not fused:
            pw = pwpool.tile([128, NTG_MAX * 128], dt.bfloat16, tag="pw")
            pw3 = pw[:].rearrange("p (t e) -> p t e", e=128)
            # Broadcast per-edge dst offsets across the 128 one-hot columns.
            nc.scalar.activation(
                out=pw3[:, :ntg, :],
                in_=offs_t[:, :ntg].unsqueeze(2).broadcast_to([128, ntg, 128]),
                func=mybir.ActivationFunctionType.Copy,
            )
            # One-hot: P[e, j] = (offs_e == j)
            nc.vector.tensor_tensor(
                out=pw3[:, :ntg, :],
                in0=iota_t[:].unsqueeze(1).broadcast_to([128, ntg, 128]),
                in1=pw3[:, :ntg, :],
                op=mybir.AluOpType.is_equal,
            )
            # msg = h[src] * w (in place on the used 32 columns)
            nc.vector.tensor_tensor(
                out=g3[:, :ntg, 0:32],
                in0=g3[:, :ntg, 0:32],
                in1=wt_t[:, :ntg].unsqueeze(2).broadcast_to([128, ntg, 32]),
                op=mybir.AluOpType.mult,
            )

        chunk_base = np.concatenate([[0], np.cumsum(ncg)]).astype(int)
        for s in range(s0, s1):
            nb = int(NB[s].sum())
            if nb == 0:
                nc.vector.memset(out_sbuf[:, s * 32:(s + 1) * 32], 0.0)
                continue
            ps = pspool.tile([128, 32], dt.float32)
            bi = 0
            for ch in range(CH):
                nbs = int(NB[s, ch])
                if nbs == 0:
                    continue
                lt0 = int(chunk_base[ch] + NB[s0:s, ch].sum())
                for b in range(nbs):
                    t = lt0 + b
                    if fused:
                        # P_w[e, j] = (offs_e == j) * w_e in one DVE op
                        pwb = pwpool.tile([128, 128], dt.bfloat16, tag="pwb")
                        nc.vector.tensor_scalar(
                            out=pwb[:],
                            in0=iota_t[:],
                            scalar1=offs_t[:, t:t + 1],
                            scalar2=wt_t[:, t:t + 1],
                            op0=mybir.AluOpType.is_equal,
                            op1=mybir.AluOpType.mult,
                        )
                        lhs = pwb[:]
                    else:
                        lhs = pw[:, t * 128:(t + 1) * 128]
                    nc.tensor.matmul(
                        out=ps[:],
                        lhsT=lhs,
                        rhs=g3[:, t, 0:32],
                        start=(bi == 0),
                        stop=(bi == nb - 1),
                    )
                    bi += 1
            nc.scalar.copy(out=out_sbuf[:, s * 32:(s + 1) * 32], in_=ps[:])
        toff += ntg


def _emit_pipeline_p4(
    nc, tile, mybir, dt, NB, c, g_ncg, NTG_MAX,
    iota_t, qiota_t, out_sbuf,
    gpool, pwpool, smpool, pspool,
    h_ext, idx_ext, offs_ext, wt_ext, q_ext,
):
    """Pack-4 pipeline: h4 table [N/4, 128] bf16 holds 4 nodes per 256B row.

    Per tile: fused one-hot P=(iota==offs)*w (DVE), quarter-select
    g3=(qiota==q)*g3 (DVE stt), matmul -> psum [v,128]=4 quarter-partials,
    per-strip fold via strided tensor_reduce.
    """
    S, CHN, G, SPG = c["S"], c["CHN"], c["G"], c["SPG"]
    subg = int(os.environ.get("K_SUBG", "0"))  # tiles per sub-gather (0=off)
    nqs = max(1, int(os.environ.get("K_SWDGEQ", "4")))
    skip_gather = bool(int(os.environ.get("K_SKIP_GATHER", "0")))
    skip_compute = bool(int(os.environ.get("K_SKIP_COMPUTE", "0")))
    qrr = bool(int(os.environ.get("K_QRR", "1")))
    qctr = [0]

    def next_q():
        q = qctr[0] % nqs
        qctr[0] += 1
        return q

    toff = 0
    for g in range(G):
        s0, s1 = g * SPG, min((g + 1) * SPG, S)
        ntg = g_ncg[g][0]
        if ntg == 0:
            for s in range(s0, s1):
                nc.vector.memset(out_sbuf[:, s * 32:(s + 1) * 32], 0.0)
            continue

        gbuf = gpool.tile([128, NTG_MAX * 128], dt.bfloat16, tag="gbuf")
        idx_t = smpool.tile([128, NTG_MAX * 8], dt.int16, tag="idx")
        offs_t = smpool.tile([128, NTG_MAX], dt.float32, tag="offs")
        wt_t = smpool.tile([128, NTG_MAX], dt.float32, tag="wt")
        q_t = smpool.tile([128, NTG_MAX], dt.float32, tag="q")

        nc.sync.dma_start(
            out=idx_t[:, : ntg * 8], in_=idx_ext[:, toff * 8:(toff + ntg) * 8]
        )
        nc.sync.dma_start(out=offs_t[:, :ntg], in_=offs_ext[:, toff:toff + ntg])
        nc.sync.dma_start(out=wt_t[:, :ntg], in_=wt_ext[:, toff:toff + ntg])
        nc.sync.dma_start(out=q_t[:, :ntg], in_=q_ext[:, toff:toff + ntg])

        g3 = gbuf[:].rearrange("p (t e) -> p t e", e=128)
        if not skip_gather:
            step = subg if subg else ntg
            for o in range(0, ntg, step):
                m = min(step, ntg - o)
                nc.gpsimd.dma_gather(
                    out_ap=g3[:, o:o + m, :],
                    in_ap=h_ext[0:CHN, :],
                    idxs_ap=idx_t[:, o * 8:(o + m) * 8],
                    num_idxs=m * 128,
                    num_idxs_reg=m * 128,
                    elem_size=128,
                    elem_step=128,
                    single_packet=(m * 128 <= 1024) if subg else False,
                    queue_num=next_q() if qrr else 0,
                )

        if skip_compute:
            for s in range(s0, s1):
                nc.vector.memset(out_sbuf[:, s * 32:(s + 1) * 32], 0.0)
            toff += ntg
            continue

        # Hoist ALL P-builds (no gather dependency) ahead of the
        # gather-dependent quarter-selects so the in-order DVE sequencer
        # isn't head-of-line blocked waiting on gather semaphores.
        pw = pwpool.tile([128, NTG_MAX * 128], dt.bfloat16, tag="pw")
        pw3 = pw[:].rearrange("p (t e) -> p t e", e=128)
        for t in range(ntg):
            # P[e, v] = (iota == offs_e) * w_e  (one DVE op, bf16)
            nc.vector.tensor_scalar(
                out=pw3[:, t, :],
                in0=iota_t[:],
                scalar1=offs_t[:, t:t + 1],
                scalar2=wt_t[:, t:t + 1],
                op0=mybir.AluOpType.is_equal,
                op1=mybir.AluOpType.mult,
            )

        for s in range(s0, s1):
            nb = int(NB[s, 0])
            if nb == 0:
                nc.vector.memset(out_sbuf[:, s * 32:(s + 1) * 32], 0.0)
                continue
            ps = pspool.tile([128, 128], dt.float32)
            lt0 = int(NB[s0:s, 0].sum())
            for b in range(nb):
                t = lt0 + b
                # quarter-select in place: g3 = (qiota == q_e) * g3
                nc.vector.scalar_tensor_tensor(
                    out=g3[:, t, :],
                    in0=qiota_t[:],
                    scalar=q_t[:, t:t + 1],
                    in1=g3[:, t, :],
                    op0=mybir.AluOpType.is_equal,
                    op1=mybir.AluOpType.mult,
                )
                nc.tensor.matmul(
                    out=ps[:],
                    lhsT=pw3[:, t, :],
                    rhs=g3[:, t, :],
                    start=(b == 0),
                    stop=(b == nb - 1),
                )
            # fold the 4 quarter partials: out[v, f] = sum_q ps[v, 32q+f]
            nc.vector.tensor_reduce(
                out=out_sbuf[:, s * 32:(s + 1) * 32],
                in_=ps[:].rearrange("p (q f) -> p f q", f=32),
                axis=mybir.AxisListType.X,
                op=mybir.AluOpType.add,
            )
        toff += ntg


def _emit_pipeline_ind(
    nc, tile, mybir, dt, NB, c, g_ncg, NTG_MAX,
    iota_t, out_sbuf,
    gpool, pwpool, smpool, pspool,
    h_ext, idx_ext, offs_ext, wt_ext,
):
    """Indirect-DMA pipeline: per-edge 64B rows h[src] gathered via the
    dynamic-DGE path (int32 offsets, one desc per edge, 16 DMA engines).

    Per tile: fused one-hot P=(iota==offs)*w (DVE), matmul [K=128e, M=128v,
    N=32f] accumulating per-strip PSUM, per-strip copy to out_sbuf.
    """
    from concourse import bass

    S, G, SPG = c["S"], c["G"], c["SPG"]
    subg = int(os.environ.get("K_SUBG", "0"))  # tiles per sub-gather (0=off)
    skip_gather = bool(int(os.environ.get("K_SKIP_GATHER", "0")))
    skip_compute = bool(int(os.environ.get("K_SKIP_COMPUTE", "0")))

    toff = 0
    for g in range(G):
        s0, s1 = g * SPG, min((g + 1) * SPG, S)
        ntg = g_ncg[g][0]
        if ntg == 0:
            for s in range(s0, s1):
                nc.vector.memset(out_sbuf[:, s * 32:(s + 1) * 32], 0.0)
            continue

        gbuf = gpool.tile([128, NTG_MAX * 32], dt.bfloat16, tag="gbuf")
        idx_t = smpool.tile([128, NTG_MAX], dt.int32, tag="idx")
        offs_t = smpool.tile([128, NTG_MAX], dt.float32, tag="offs")
        wt_t = smpool.tile([128, NTG_MAX], dt.float32, tag="wt")

        nc.sync.dma_start(out=idx_t[:, :ntg], in_=idx_ext[:, toff:toff + ntg])
        nc.sync.dma_start(out=offs_t[:, :ntg], in_=offs_ext[:, toff:toff + ntg])
        nc.sync.dma_start(out=wt_t[:, :ntg], in_=wt_ext[:, toff:toff + ntg])

        g3 = gbuf[:].rearrange("p (t e) -> p t e", e=32)
        if not skip_gather:
            step = subg if subg else ntg
            for o in range(0, ntg, step):
                m = min(step, ntg - o)
                nc.gpsimd.indirect_dma_start(
                    out=g3[:, o:o + m, :],
                    out_offset=None,
                    in_=h_ext[:],
                    in_offset=bass.IndirectOffsetOnAxis(
                        ap=idx_t[:, o:o + m], axis=0
                    ),
                )

        if skip_compute:
            for s in range(s0, s1):
                nc.vector.memset(out_sbuf[:, s * 32:(s + 1) * 32], 0.0)
            toff += ntg
            continue

        for s in range(s0, s1):
            nb = int(NB[s, 0])
            if nb == 0:
                nc.vector.memset(out_sbuf[:, s * 32:(s + 1) * 32], 0.0)
                continue
            ps = pspool.tile([128, 32], dt.float32)
            lt0 = int(NB[s0:s, 0].sum())
            for b in range(nb):
                t = lt0 + b
                # P[e, v] = (iota == offs_e) * w_e  (one DVE op, bf16)
                pwb = pwpool.tile([128, 128], dt.bfloat16, tag="pwb")
                nc.vector.tensor_scalar(
                    out=pwb[:],
                    in0=iota_t[:],
                    scalar1=offs_t[:, t:t + 1],
                    scalar2=wt_t[:, t:t + 1],
                    op0=mybir.AluOpType.is_equal,
                    op1=mybir.AluOpType.mult,
                )
                nc.tensor.matmul(
                    out=ps[:],
                    lhsT=pwb[:],
                    rhs=g3[:, t, :],
                    start=(b == 0),
                    stop=(b == nb - 1),
                )
            nc.scalar.copy(out=out_sbuf[:, s * 32:(s + 1) * 32], in_=ps[:])
        toff += ntg


def _make_in_maps(h, c, NB, idx_wrapped, offs_arr, w_arr, q_arr):
    N, D, NC = c["N"], c["D"], c["NC"]
    iota = np.broadcast_to(
        np.arange(128, dtype=np.float32).astype(BF16), (128, 128)
    ).copy()
    if c["IND"]:
        h_bf = np.ascontiguousarray(np.asarray(h, dtype=np.float32).astype(BF16))
        return [
            {
                "h": h_bf,
                "idx": idx_wrapped[i],
                "offs": offs_arr[i],
                "wt": w_arr[i],
                "iota": iota,
            }
            for i in range(NC)
        ]
    if c["PACK4"]:
        h4 = np.asarray(h, dtype=np.float32).astype(BF16)
        assert h4.shape == (N, D) and D * 4 == 128
        h4 = np.ascontiguousarray(h4.reshape(N // 4, 128))
        qiota = np.broadcast_to(
            (np.arange(128) // 32).astype(np.float32).astype(BF16), (128, 128)
        ).copy()
        return [
            {
                "h": h4,
                "idx": np.ascontiguousarray(np.tile(idx_wrapped[i], (8, 1))),
                "offs": offs_arr[i],
                "wt": w_arr[i],
                "q": q_arr[i],
                "iota": iota,
                "qiota": qiota,
            }
            for i in range(NC)
        ]
    h_pad = np.zeros((N, 128), dtype=BF16)
    h_pad[:, :D] = np.asarray(h, dtype=np.float32).astype(BF16)
    return [
        {
            "h": h_pad,
            "idx": np.ascontiguousarray(np.tile(idx_wrapped[i], (8, 1))),
            "offs": offs_arr[i],
            "wt": w_arr[i],
            "iota": iota,
        }
        for i in range(NC)
    ]


def run_cfg(h, w, src, dst, cfg, trace=False):
    from concourse.bass_utils import run_bass_kernel_spmd

    c = _derive(cfg)
    N, D, NC, NPC, S = c["N"], c["D"], c["NC"], c["NPC"], c["S"]

    NB, idx_wrapped, offs_arr, w_arr, q_arr = _plan(src, dst, w, c)
    nc = _build(NB, c)

    in_maps = _make_in_maps(h, c, NB, idx_wrapped, offs_arr, w_arr, q_arr)
    res = run_bass_kernel_spmd(nc, in_maps, list(range(NC)), trace=trace)
    out = np.empty((N, D), dtype=np.float32)
    for i in range(NC):
        out[i * NPC:(i + 1) * NPC] = res.results[i]["out"][:NPC]
    return out, res


def make_runner(h, w, src, dst, cfg):
    """Build a reusable jitted SPMD callable for timing: returns
    (run_once, assemble) where run_once() returns unblocked device arrays."""
    import jax
    import jax.numpy as jnp
    from jax.sharding import Mesh, PartitionSpec, NamedSharding
    from jax.experimental.shard_map import shard_map
    from concourse import bass2jax, mybir

    c = _derive(cfg)
    N, D, NC, NPC = c["N"], c["D"], c["NC"], c["NPC"]

    NB, idx_wrapped, offs_arr, w_arr, q_arr = _plan(src, dst, w, c)
    nc = _build(NB, c)

    in_maps = _make_in_maps(h, c, NB, idx_wrapped, offs_arr, w_arr, q_arr)

    bass2jax.install_neuronx_cc_hook()
    partition_name = nc.partition_id_tensor.name if nc.partition_id_tensor else None
    in_names, out_names, out_avals, zero_shapes = [], [], [], []
    for alloc in nc.m.functions[0].allocations:
        if not isinstance(alloc, mybir.MemoryLocationSet):
            continue
        name = alloc.memorylocations[0].name
        if alloc.kind == "ExternalInput":
            if name != partition_name:
                in_names.append(name)
        elif alloc.kind == "ExternalOutput":
            out_names.append(name)
            shape = tuple(alloc.tensor_shape)
            dtype = mybir.dt.np(alloc.dtype)
            out_avals.append(jax.core.ShapedArray(shape, dtype))
            zero_shapes.append((shape, dtype))
    n_params = len(in_names)
    n_outs = len(out_avals)
    all_in_names = list(in_names) + list(out_names)
    if partition_name is not None:
        all_in_names.append(partition_name)

    def _body(*args):
        operands = list(args)
        if partition_name is not None:
            operands.append(bass2jax.partition_id_tensor())
        outs = bass2jax._bass_exec_p.bind(
            *operands,
            out_avals=tuple(out_avals),
            in_names=tuple(all_in_names),
            out_names=tuple(out_names),
            lowering_input_output_aliases=(),
            sim_require_finite=True,
            sim_require_nnan=True,
            nc=nc,
        )
        return tuple(outs)

    devices = jax.devices()[:NC]
    mesh = Mesh(np.asarray(devices), ("core",))
    donate = tuple(range(n_params, n_params + n_outs))
    sharded = jax.jit(
        shard_map(
            _body,
            mesh=mesh,
            in_specs=(PartitionSpec("core"),) * (n_params + n_outs),
            out_specs=(PartitionSpec("core"),) * n_outs,
            check_rep=False,
        ),
        donate_argnums=donate,
        keep_unused=True,
    )

    concat_in = [
        np.concatenate([np.asarray(in_maps[k][nm]) for k in range(NC)], axis=0)
        for nm in in_names
    ]
    shard = NamedSharding(mesh, PartitionSpec("core"))
    dev_in = [jax.device_put(a, shard) for a in concat_in]

    zeros_fn = jax.jit(
        lambda: tuple(
            jnp.zeros((NC * s[0], *s[1:]), dt) for (s, dt) in zero_shapes
        ),
        out_shardings=(shard,) * n_outs,
    )

    def run_once():
        zs = zeros_fn()
        return sharded(*dev_in, *zs)

    def assemble(out_arrs):
        full = np.empty((N, D), dtype=np.float32)
        o = np.asarray(out_arrs[0]).reshape(NC, -1, D)
        for i in range(NC):
            full[i * NPC:(i + 1) * NPC] = o[i, :NPC]
        return full

    # chained executor: K back-to-back executions in ONE dispatch, each
    # feeding its output as the next call's out-operand (defeats CSE).
    def make_chain(k):
        def _chain_body(*args):
            ins, outs = args[:n_params], list(args[n_params:])
            for _ in range(k):
                outs = list(_body(*ins, *outs))
            return tuple(outs)

        return jax.jit(
            shard_map(
                _chain_body,
                mesh=mesh,
                in_specs=(PartitionSpec("core"),) * (n_params + n_outs),
                out_specs=(PartitionSpec("core"),) * n_outs,
                check_rep=False,
            ),
            donate_argnums=donate,
            keep_unused=True,
        )

    def run_chain(chain_fn):
        zs = zeros_fn()
        return chain_fn(*dev_in, *zs)

    return run_once, assemble, make_chain, run_chain


def kernel(**inputs):
    out, _ = run_cfg(
        inputs["h"], inputs["w"], inputs["src"], inputs["dst"], FULL_CFG
    )
    return out

